# revision 1
# baseline (speedup 1.0000x reference)
"""Trainium2 Bass kernel for the HardResetSSMBlock problem.

y = silu(x @ W1 + b1) @ W2 + b2, masked per frame, with a periodic decay
scale on frames where (t+1) % 10 == 0.

Strategy: data-parallel over 8 NeuronCores (2 batch rows each -> 32768
tokens per core). The host feeds x pre-transposed into [n_tiles, 128, 512]
feature-major tiles, so each 512-token tile runs:
  DMA in (contiguous) -> MM1 (W1 stationary, X^T moving) -> Silu(+b1) on
  ACT writing float32r -> MM2 with H^T chunks as the stationary operand so
  Y lands token-major -> fused (mask * decay) scale via broadcast
  tensor_mul on DVE -> DMA out.

The per-token scale s = mask * decay is precomputed on host (a cheap [B,S]
elementwise product) and fed pre-transposed as [128, n_chunks] so it can
be applied as per-partition scalars on the token-major output.
"""

import numpy as np

B, S, D = 16, 16384, 128
RESET_PERIOD = 10
DECAY_FACTOR = 0.1
N_CORES = 8
TOK_PER_CORE = B * S // N_CORES  # 32768
TILE_TOK = 2048
CH = TILE_TOK // 128  # 4 chunks of 128 tokens
N_TILES = TOK_PER_CORE // TILE_TOK  # 64

# float32r matmuls stream 1 col/cycle on HW (vs 4 for fp32). f32r is a
# distinct rounding: a compute engine must produce the operand. MM2's
# stationary+moving come from on-chip ops, so it can use f32r cheaply
# (silu writes f32r; W2 converted once). MM1 reads x straight from DMA:
# if MM1_DMA_F32R, the DMA destination is declared f32r (numerics verified
# against fp32 on HW); otherwise MM1 runs plain fp32.
MM1_DMA_F32R = True
MM2_F32R = True
# Silu on hardware; CoreSim lacks it, so sim tests may override (e.g. Sigmoid)
ACT_FUNC = "Silu"

_CACHE = {}


def _build_nc():
    import concourse.bacc as bacc
    import concourse.tile as tile
    from concourse import mybir
    from concourse.bass import AP

    f32 = mybir.dt.float32
    f32r = mybir.dt.float32r

    nc = bacc.Bacc()
    xt_d = nc.dram_tensor(
        "x_t", [N_TILES, 128, TILE_TOK],
        f32r if MM1_DMA_F32R else f32, kind="ExternalInput"
    )
    st_d = nc.dram_tensor(
        "s_t", [128, TOK_PER_CORE // 128], f32, kind="ExternalInput"
    )
    w1_d = nc.dram_tensor("w1", [D, D], f32, kind="ExternalInput")
    w2_d = nc.dram_tensor("w2", [D, D], f32, kind="ExternalInput")
    b1_d = nc.dram_tensor("b1", [D, 1], f32, kind="ExternalInput")
    # partition-major output tiles: y_t[t, p, c, d] = y[(t*CH + c)*128 + p, d]
    # (host un-permutes; this makes the out-DMA write 2KB contiguous runs)
    y_d = nc.dram_tensor(
        "y_t", [N_TILES, 128, CH, D], f32, kind="ExternalOutput"
    )

    mm1_in_dt = f32r if MM1_DMA_F32R else f32
    mm2_dt = f32r if MM2_F32R else f32

    with tile.TileContext(nc) as tc:
        with (
            tc.tile_pool(name="const", bufs=1) as constp,
            tc.tile_pool(name="xt", bufs=6) as xtp,
            tc.tile_pool(name="ht", bufs=4) as htp,
            tc.tile_pool(name="yout", bufs=4) as youtp,
            tc.tile_pool(name="ps_ht", bufs=4, space="PSUM") as ps_htp,
            tc.tile_pool(name="ps_y", bufs=4, space="PSUM") as ps_yp,
        ):
            w1_raw = constp.tile([128, 128], f32)
            nc.gpsimd.dma_start(w1_raw[:], w1_d[:])
            if MM1_DMA_F32R:
                w1_s = constp.tile([128, 128], f32r, tag="w1r")
                nc.vector.tensor_copy(w1_s[:], w1_raw[:])
            else:
                w1_s = w1_raw
            w2_raw = constp.tile([128, 128], f32)
            nc.gpsimd.dma_start(w2_raw[:], w2_d[:])
            if MM2_F32R:
                w2_s = constp.tile([128, 128], f32r, tag="w2r")
                nc.vector.tensor_copy(w2_s[:], w2_raw[:])
            else:
                w2_s = w2_raw
            b1_s = constp.tile([128, 1], f32)
            nc.gpsimd.dma_start(b1_s[:], b1_d[:])
            st_s = constp.tile([128, TOK_PER_CORE // 128], f32)
            nc.gpsimd.dma_start(st_s[:], st_d[:])



            for t in range(N_TILES):
                s_xt = xtp.tile([128, TILE_TOK], mm1_in_dt)
                in_eng = nc.sync if t % 2 == 0 else nc.scalar
                in_eng.dma_start(s_xt[:], xt_d[t])

                s_ht = htp.tile([128, TILE_TOK], mm2_dt)
                s_y = youtp.tile([128, CH, 128], f32)
                for h in range(TILE_TOK // 512):
                    hs = slice(h * 512, (h + 1) * 512)
                    p_ht = ps_htp.tile([128, 512], f32)
                    nc.tensor.matmul(
                        p_ht[:], w1_s[:], s_xt[:, hs], start=True, stop=True
                    )
                    nc.scalar.activation(
                        s_ht[:, hs], p_ht[:],
                        getattr(mybir.ActivationFunctionType, ACT_FUNC),
                        bias=b1_s[:], scale=1.0,
                    )
                    p_y = ps_yp.tile([128, 4, 128], f32)
                    for c in range(4):
                        cc = h * 4 + c
                        nc.tensor.matmul(
                            p_y[:, c, :],
                            s_ht[:, cc * 128:(cc + 1) * 128], w2_s,
                            start=True, stop=True,
                        )
                    s_slice = st_s[:, t * CH + h * 4:t * CH + h * 4 + 4]
                    s_bcast = AP(
                        tensor=s_slice.tensor,
                        offset=s_slice.offset,
                        ap=list(s_slice.ap) + [[0, 128]],
                    )  # [128, 4, 128] with stride-0 feature dim
                    nc.vector.tensor_mul(
                        s_y[:, h * 4:(h + 1) * 4, :], p_y[:], s_bcast
                    )

                out_eng = nc.scalar if t % 2 == 0 else nc.sync
                out_eng.dma_start(y_d[t], s_y[:])

    nc.finalize()
    return nc


def _get_nc():
    if "nc" not in _CACHE:
        _CACHE["nc"] = _build_nc()
    return _CACHE["nc"]


def _host_prep(x, mask, W1, b1, W2, b2):
    """Shard inputs across 8 cores; pre-transpose x; per-token scale."""
    x = np.asarray(x, dtype=np.float32)
    mask = np.asarray(mask)
    W1 = np.ascontiguousarray(np.asarray(W1, dtype=np.float32))
    W2 = np.ascontiguousarray(np.asarray(W2, dtype=np.float32))
    b1 = np.asarray(b1, dtype=np.float32).reshape(D, 1)

    t = np.arange(S)
    decay = np.where((t + 1) % RESET_PERIOD == 0, DECAY_FACTOR, 1.0).astype(
        np.float32
    )
    s = mask.astype(np.float32) * decay[None, :]  # [B, S]

    # [B*S, D] -> per-core [N_TILES, D, TILE_TOK] feature-major tiles
    x_t_all = np.ascontiguousarray(
        x.reshape(N_CORES, N_TILES, TILE_TOK, D).transpose(0, 1, 3, 2)
    )

    rows_per_core = B // N_CORES
    in_maps = []
    for c in range(N_CORES):
        ss = s[c * rows_per_core:(c + 1) * rows_per_core].reshape(TOK_PER_CORE)
        s_t = np.ascontiguousarray(
            ss.reshape(TOK_PER_CORE // 128, 128).T
        )  # [128, n_chunks]
        in_maps.append(
            {
                "x_t": x_t_all[c],
                "s_t": s_t,
                "w1": W1,
                "w2": W2,
                "b1": b1,
            }
        )
    return in_maps


def kernel(x, mask, W1, b1, W2, b2, _trace=False):
    from concourse.bass_utils import run_bass_kernel_spmd

    b2 = np.asarray(b2, dtype=np.float32)

    nc = _get_nc()
    in_maps = _host_prep(x, mask, W1, b1, W2, b2)
    res = run_bass_kernel_spmd(
        nc, in_maps, list(range(N_CORES)), trace=_trace
    )
    if _trace:
        _CACHE["last_results"] = res
    # y_t[t, p, c, d] -> y[(t*CH + c)*128 + p, d]
    out = np.stack([res.results[c]["y_t"] for c in range(N_CORES)])
    out = np.ascontiguousarray(out.transpose(0, 1, 3, 2, 4)).reshape(B, S, D)
    if np.any(b2):
        # device computes (h @ W2) * s; the masked/decayed bias is added here
        t = np.arange(S)
        decay = np.where(
            (t + 1) % RESET_PERIOD == 0, DECAY_FACTOR, 1.0
        ).astype(np.float32)
        s = np.asarray(mask).astype(np.float32) * decay[None, :]
        out = out + s[:, :, None] * b2[None, None, :]
    return out



# revision 2
# speedup vs baseline: 2.3948x; 2.3948x over previous
"""Trainium2 Bass kernel for the HardResetSSMBlock problem.

y = silu(x @ W1 + b1) @ W2 + b2, masked per frame, with a periodic decay
scale on frames where (t+1) % 10 == 0.

The op is memory-bound: 134 MB in + 134 MB out at f32. Two structural
cuts get the device traffic down ~4x vs the dense-f32 version:

1. The mask zeroes ~half the frames, and the host knows the mask. Only
   unmasked tokens are packed (host gather) and shipped; the result is
   scattered back into a zeros array on host. The per-token decay scale
   is likewise applied during the host-side scatter, so the device needs
   no mask/scale input at all.
2. bf16 on the wire in both directions (and for the weights). Verified
   rel-err of the bf16 pipeline vs the f32 reference is ~4e-3 against a
   tolerance of 2e-2.

Device program per 2048-token tile (8 cores, data-parallel over packed
tokens; tile count adapts to the mask popcount, cached per NEFF):
  DMA in (x^T, feature-major bf16) -> 4x matmul [128,512] with W1
  stationary -> Silu(+b1) on ACT in 1024-col halves (f32 PSUM -> bf16
  SBUF) -> 4x matmul with W2 stationary (y stays feature-major, no
  per-chunk stationary reloads) -> DVE copy PSUM -> bf16 SBUF -> DMA out.
PSUM: 2 bufs x [128,1024] for MM1 + 2 x [128,1024] for MM2 = all 8 banks.
"""

import numpy as np

B, S, D = 16, 16384, 128
RESET_PERIOD = 10
DECAY_FACTOR = 0.1
N_CORES = 8
TILE_TOK = 2048
BLK = 512   # one matmul output = one PSUM bank
HALF = 1024  # ACT / DVE instruction granularity (2 banks)

ACT_FUNC = "Silu"

_CACHE = {}


def _build_nc(n_tiles):
    import concourse.bacc as bacc
    import concourse.tile as tile
    from concourse import mybir

    f32 = mybir.dt.float32
    bf16 = mybir.dt.bfloat16

    nc = bacc.Bacc()
    xt_d = nc.dram_tensor(
        "x_t", [n_tiles, 128, TILE_TOK], bf16, kind="ExternalInput"
    )
    w1_d = nc.dram_tensor("w1", [D, D], bf16, kind="ExternalInput")
    w2_d = nc.dram_tensor("w2", [D, D], bf16, kind="ExternalInput")
    b1_d = nc.dram_tensor("b1", [D, 1], f32, kind="ExternalInput")
    # feature-major output tiles: y_t[t, e, j] = y[t*TILE_TOK + j, e]
    y_d = nc.dram_tensor(
        "y_t", [n_tiles, 128, TILE_TOK], bf16, kind="ExternalOutput"
    )

    act = getattr(mybir.ActivationFunctionType, ACT_FUNC)

    with tile.TileContext(nc) as tc:
        with (
            tc.tile_pool(name="const", bufs=1) as constp,
            tc.tile_pool(name="xt", bufs=4) as xtp,
            tc.tile_pool(name="ht", bufs=3) as htp,
            tc.tile_pool(name="yout", bufs=4) as youtp,
            tc.tile_pool(name="ps_h", bufs=2, space="PSUM") as ps_hp,
            tc.tile_pool(name="ps_y", bufs=2, space="PSUM") as ps_yp,
        ):
            w1_s = constp.tile([128, 128], bf16)
            nc.gpsimd.dma_start(w1_s[:], w1_d[:])
            w2_s = constp.tile([128, 128], bf16)
            nc.gpsimd.dma_start(w2_s[:], w2_d[:])
            b1_s = constp.tile([128, 1], f32)
            nc.gpsimd.dma_start(b1_s[:], b1_d[:])

            for t in range(n_tiles):
                s_xt = xtp.tile([128, TILE_TOK], bf16)
                in_eng = nc.sync if t % 2 == 0 else nc.scalar
                in_eng.dma_start(s_xt[:], xt_d[t])

                s_ht = htp.tile([128, TILE_TOK], bf16)
                s_y = youtp.tile([128, TILE_TOK], bf16)
                for hf in range(TILE_TOK // HALF):
                    p_h = ps_hp.tile([128, HALF], f32)
                    for bk in range(HALF // BLK):
                        lo = hf * HALF + bk * BLK
                        nc.tensor.matmul(
                            p_h[:, bk * BLK:(bk + 1) * BLK],
                            w1_s[:], s_xt[:, lo:lo + BLK],
                            start=True, stop=True,
                        )
                    nc.scalar.activation(
                        s_ht[:, hf * HALF:(hf + 1) * HALF], p_h[:],
                        act, bias=b1_s[:], scale=1.0,
                    )
                for hf in range(TILE_TOK // HALF):
                    p_y = ps_yp.tile([128, HALF], f32)
                    for bk in range(HALF // BLK):
                        lo = hf * HALF + bk * BLK
                        nc.tensor.matmul(
                            p_y[:, bk * BLK:(bk + 1) * BLK],
                            w2_s[:], s_ht[:, lo:lo + BLK],
                            start=True, stop=True,
                        )
                    nc.vector.tensor_copy(
                        s_y[:, hf * HALF:(hf + 1) * HALF], p_y[:]
                    )

                out_eng = nc.scalar if t % 2 == 0 else nc.sync
                out_eng.dma_start(y_d[t], s_y[:])

    nc.finalize()
    return nc


def _get_nc(n_tiles):
    key = ("nc", n_tiles)
    if key not in _CACHE:
        _CACHE[key] = _build_nc(n_tiles)
    return _CACHE[key]


def kernel(x, mask, W1, b1, W2, b2, _trace=False):
    from concourse import mybir
    from concourse.bass_utils import run_bass_kernel_spmd

    bf16 = mybir.dt.np(mybir.dt.bfloat16)

    x = np.asarray(x, dtype=np.float32)
    mask = np.asarray(mask).astype(bool)
    W1 = np.asarray(W1, dtype=np.float32)
    W2 = np.asarray(W2, dtype=np.float32)
    b1 = np.asarray(b1, dtype=np.float32)
    b2 = np.asarray(b2, dtype=np.float32)

    Bx, Sx, Dx = x.shape
    x_flat = np.ascontiguousarray(x).reshape(Bx * Sx, Dx)
    idx = np.flatnonzero(mask.ravel())
    count = idx.size

    t = np.arange(Sx)
    decay = np.where(
        (t + 1) % RESET_PERIOD == 0, DECAY_FACTOR, 1.0
    ).astype(np.float32)

    out = np.zeros((Bx * Sx, Dx), dtype=np.float32)

    if count > 0:
        n_tiles = -(-count // (N_CORES * TILE_TOK))
        cap = n_tiles * TILE_TOK
        total = cap * N_CORES

        xg = np.zeros((total, Dx), dtype=bf16)
        xg[:count] = x_flat[idx].astype(bf16)
        # per-core feature-major tiles [n_tiles, D, TILE_TOK]
        x_t = np.ascontiguousarray(
            xg.reshape(N_CORES, n_tiles, TILE_TOK, Dx).transpose(0, 1, 3, 2)
        )

        w1b = np.ascontiguousarray(W1.astype(bf16))
        w2b = np.ascontiguousarray(W2.astype(bf16))
        b1c = np.ascontiguousarray(b1.reshape(Dx, 1))

        nc = _get_nc(n_tiles)
        in_maps = [
            {"x_t": x_t[c], "w1": w1b, "w2": w2b, "b1": b1c}
            for c in range(N_CORES)
        ]
        res = run_bass_kernel_spmd(
            nc, in_maps, list(range(N_CORES)), trace=_trace
        )
        if _trace:
            _CACHE["last_results"] = res

        # y_t [n_tiles, D, TILE_TOK] feature-major -> token-major rows
        y_all = np.stack([res.results[c]["y_t"] for c in range(N_CORES)])
        y_tok = np.ascontiguousarray(
            y_all.transpose(0, 1, 3, 2)
        ).reshape(total, Dx)
        dec_flat = np.broadcast_to(decay, (Bx, Sx)).reshape(-1)
        out[idx] = y_tok[:count].astype(np.float32) * dec_flat[idx][:, None]

    out = out.reshape(Bx, Sx, Dx)
    if np.any(b2):
        # device computes h @ W2 without b2; reference adds b2 before the
        # mask/decay scaling, so fold it in here on the host
        s_full = mask.astype(np.float32) * decay[None, :]
        out = out + s_full[:, :, None] * b2[None, None, :]
    return out
